# revision 24
# baseline (speedup 1.0000x reference)
"""BERT self-attention (B=8, S=1024, D=768, H=12) on 8 TRN2 NeuronCores.

Strategy
--------
Data-parallel over batch: core b handles batch element b (no collectives).

Per core (layouts keep the contraction dim in the partition axis):

  1. mixT[e, s] = sum_d W^T[d, e] * x^T[d, s] + bias[e]: bf16 matmuls,
     fp32 psum, bias added during the DVE evacuation to bf16 mixbf.
     Input DMAs are (w0 slice, x chunk) pairs round-robined k-ordered
     across the 3 DMA queues; pair-0 projection is emitted k-major with
     warmup transposes filling PE gaps, so matmuls fire as chunks land
     and the PE clock ramps. w1 streams last.
  2. Q=K=V => scores are symmetric; the exp'd tile in [t, s] layout equals
     the transposed (unnormalized) probability matrix. scores chunk =
     mixbf[64q:64q+64, j]^T @ mixbf[64q:64q+64, j] (64-partition matmuls,
     no zero-masking copies), fp32 psum. Band-limited symmetry (R=2):
     chunk i only computes/exps columns >= 128*(i-R); the skipped tiles
     are EXACT PE-transposes of their symmetric counterparts.
  3. exp: 84 ACTIVATEs (banded widths; chunks 6+7 share one) -> u tiles
     bf16 in SBUF. The ACT engine is ~100% saturated in steady state and
     paces the kernel.
  4. ctx with U as the STATIONARY operand: out[s, dh] = sum_t U[t,s]*xl[t,dh]
     via matmul(lhsT=U(i, sc-chunk)[128,128], rhs=xl_i[128, 65]) accumulating
     per-half psum tiles [128, 4, 65] (ones column -> denominator col 64).
  5. mask folded EXACTLY into xl rows scaled by emask[t] = exp(mask[t]).
  6. Epilogue per half-head: one strided [128, 4] denominator extract, one
     reciprocal, ONE merged tensor_mul [128,4,64] (broadcast rcp)
     normalization straight into the bf16 staging tile.
  7. Output flushed as bf16 (host upcasts to fp32), grouped DMAs.

Scheduling: ACT (softmax exp) is the steady-state pacer. Emission
order: scores+exp of head h before ctx of head h-1 (deferred-ctx), the
projection/prep of pair j+1 between them, and the final head's ctx
+ epilogue trailing the last ACTIVATE.

Measured on TRN2 (8 cores): ~116.4 us HW exec (prior session ~132.0),
rel err ~6.9e-3 vs the fp32 reference. Note ~±1us run-to-run variance,
with occasional ~+20us outlier windows on a cold/busy device.
"""

import numpy as np

import concourse.bacc as bacc
import concourse.tile as tile
from concourse import mybir
from concourse.bass_utils import run_bass_kernel_spmd

B, S, D = 8, 1024, 768
H, DH = 12, 64
NP = 6            # e-tile pairs (2 heads each)
NT = 8            # t-chunks / s-chunks of 128
R = 2             # symmetry band: exp computes cols >= 128*(i-R) of chunk i
C0 = [128 * max(0, i - R) for i in range(NT)]
C0[7] = C0[6]     # chunks 6+7 share a band so their exps merge into one
                  # ACTIVATE (the extra 128 cols cost less than an ACT
                  # instruction's fixed overhead)
# transposed-tile index: tile (i, c) comes from a PE transpose of source
# chunk c's columns whenever exp skipped it (128*c < C0[i]); ctx slot sc
# with source chunk i uses IDX[(sc, i)]
IDX = {}
for _c in range(NT):
    for _i in range(NT):
        if 128 * _c < C0[_i]:
            IDX[(_c, _i)] = len(IDX)
NTR = len(IDX)    # 14
F32 = mybir.dt.float32
BF16 = mybir.dt.bfloat16
EXP = mybir.ActivationFunctionType.Exp
IDENT = mybir.ActivationFunctionType.Identity

_CACHED_NC = None


def build_nc():
    nc = bacc.Bacc("TRN2", target_bir_lowering=False)

    # Inputs are host-repacked so every SBUF partition's data is one
    # contiguous DRAM run (big DMA bursts instead of 2KB packets):
    # xr[p, k*S+s] = x[s, 128k+p], w0/w1 likewise for W^T columns 0:128
    # (the pair-0 block, loaded first) and 128:768.
    xr = nc.dram_tensor("xr", [128, NP * S], BF16, kind="ExternalInput")
    w0 = nc.dram_tensor("w0", [128, NP * 128], BF16, kind="ExternalInput")
    w1 = nc.dram_tensor("w1", [128, NP * 640], BF16, kind="ExternalInput")
    bias_d = nc.dram_tensor("bias_d", [128, NP], F32, kind="ExternalInput")
    mask_d = nc.dram_tensor("mask_d", [128, NT], F32, kind="ExternalInput")
    ident_d = nc.dram_tensor("ident_d", [128, 128], BF16, kind="ExternalInput")
    out_d = nc.dram_tensor("out", [S, D], BF16, kind="ExternalOutput")

    with tile.TileContext(nc) as tc:
        with (
            tc.tile_pool(name="consts", bufs=1) as consts,
            tc.tile_pool(name="big", bufs=1) as big,
            tc.tile_pool(name="upool", bufs=18) as upool,
            tc.tile_pool(name="rpool", bufs=8) as rpool,
            tc.tile_pool(name="utpool", bufs=2) as utpool,
            tc.tile_pool(name="ps_big", bufs=2, space="PSUM") as ps_big,
            tc.tile_pool(name="ps_ctx", bufs=2, space="PSUM") as ps_ctx,
            tc.tile_pool(name="ps_sm", bufs=1, space="PSUM") as ps_sm,
        ):
            identbf = consts.tile([128, 128], BF16)
            bias_t = consts.tile([128, NP], F32)
            mask_t = consts.tile([128, NT], F32)

            wts0 = big.tile([128, NP, 128], BF16)
            wts1 = big.tile([128, NP, 640], BF16)
            xts = big.tile([128, NP, S], BF16)

            # Tiny consts first on their queues, then per-k (w0 slice, x
            # chunk) pairs round-robined over the 3 DMA queues so the
            # k-major projection matmuls fire as chunks land; w1 (needed
            # only from prep(1)) streams last.
            nc.sync.dma_start(out=wts0[:, :, :], in_=w0[:, :])
            nc.scalar.dma_start(out=identbf, in_=ident_d[:, :])
            nc.gpsimd.dma_start(out=bias_t, in_=bias_d[:, :])
            nc.gpsimd.dma_start(out=mask_t, in_=mask_d[:, :])
            xqs = [nc.scalar, nc.gpsimd, nc.sync]
            for k in range(NP):
                xqs[k % 3].dma_start(
                    out=xts[:, k, :], in_=xr[:, k * S:(k + 1) * S])
            nc.sync.dma_start(out=wts1[:, 0:2, :], in_=w1[:, 0:2 * 640])
            nc.scalar.dma_start(out=wts1[:, 2:4, :], in_=w1[:, 2 * 640:4 * 640])
            nc.gpsimd.dma_start(out=wts1[:, 4:6, :], in_=w1[:, 4 * 640:6 * 640])

            def warmup(n):
                # Keep the PE clock ramping while inputs stream in.
                for w in range(n):
                    ptw = ps_sm.tile([128, 128], BF16, name="pt", bufs=1)
                    nc.tensor.transpose(ptw, identbf, identbf)

            warmup(6)

            # emask[t] = exp(mask[t]); folded into xl rows (exact mask).
            emask = consts.tile([128, NT], F32)
            nc.scalar.activation(out=emask, in_=mask_t, func=EXP)

            mixbf = big.tile([128, NP, S], BF16)
            stages = big.tile([128, NT, H, DH], BF16)

            # xl[t, i, q, 0:64] = emask[t] * v values; col 64 = emask[t]
            # (the ones column pre-scaled by the mask factor). Ping-pong per
            # head pair: the deferred ctx of pair j reads its slot while
            # prep(j+1) writes the other.
            xlts = [big.tile([128, NT, 2, DH + 1], BF16, name=f"xl{p}")
                    for p in range(2)]
            for xlt in xlts:
                # cols 0:64 are fully overwritten by the prep evacuations;
                # only the masked ones column needs initialization.
                nc.vector.tensor_copy(
                    out=xlt[:, :, :, DH],
                    in_=emask[:, :, None].broadcast_to((128, NT, 2)),
                )

            # Persistent ping-pong Z tiles; zero halves are set once.
            zt = [[big.tile([128, S], BF16, name=f"z{q}{p}") for p in range(2)]
                  for q in range(2)]
            for q in range(2):
                olo = (1 - q) * 64
                for p in range(2):
                    nc.gpsimd.memset(zt[q][p][olo:olo + 64, :], 0.0)

            # Preload the ACT exp table while the inputs stream in.
            warm = consts.tile([128, 8], F32)
            nc.scalar.activation(out=warm, in_=mask_t[:, 0:8], func=EXP,
                                 scale=0.125)

            def prep_proj(j):
                """Projection of head pair j into mixbf[:, j, :]."""
                if j == 0:
                    # k-major in the two (idle) ctx psum banks: both halves'
                    # k-th matmuls fire the moment x chunk k lands, with
                    # warmup transposes filling the PE gaps between chunks.
                    pms = [ps_ctx.tile([128, 4, 128], F32, name="pc")
                           for n in range(2)]
                    for k in range(NP):
                        for n in range(2):
                            nc.tensor.matmul(
                                pms[n][:, :, :],
                                lhsT=wts0[:, k, :],
                                rhs=xts[:, k, n * 512:(n + 1) * 512]
                                .rearrange("p (a b) -> p a b", a=4),
                                start=(k == 0),
                                stop=(k == NP - 1),
                            )
                        if k < NP - 1:
                            warmup(2)
                    # n=0 evacuated by the (idle) ACT engine in parallel
                    # with DVE's n=1 evac: identity is in the exp table, so
                    # no ACT table reload.
                    nc.scalar.activation(
                        out=mixbf[:, j, 0:512],
                        in_=pms[0].rearrange("p a b -> p (a b)"),
                        func=IDENT, bias=bias_t[:, j:j + 1],
                    )
                    # z(n=0) copies read the projection psum directly
                    # (bias added in-flight) so they overlap the ACT evac
                    # instead of waiting for it.
                    for q in range(2):
                        lo = q * 64
                        nc.vector.tensor_scalar_add(
                            zt[q][j % 2][lo:lo + 64, 0:512],
                            pms[0].rearrange("p a b -> p (a b)")[lo:lo + 64, :],
                            bias_t[lo:lo + 64, j:j + 1],
                        )
                    nc.vector.tensor_scalar_add(
                        mixbf[:, j, 512:1024],
                        pms[1].rearrange("p a b -> p (a b)"),
                        bias_t[:, j:j + 1],
                    )
                    for q in range(2):
                        lo = q * 64
                        nc.vector.tensor_copy(
                            out=zt[q][j % 2][lo:lo + 64, 512:1024],
                            in_=mixbf[lo:lo + 64, j, 512:1024],
                        )
                    return
                for n in range(2):
                    pm = ps_sm.tile([128, 512], F32, name="pm", bufs=1)
                    for k in range(NP):
                        nc.tensor.matmul(
                            pm,
                            lhsT=wts1[:, k, (j - 1) * 128:j * 128],
                            rhs=xts[:, k, n * 512:(n + 1) * 512],
                            start=(k == 0),
                            stop=(k == NP - 1),
                        )
                    nc.vector.tensor_scalar_add(
                        mixbf[:, j, n * 512:(n + 1) * 512], pm,
                        bias_t[:, j:j + 1]
                    )
                    for q in range(2):
                        lo = q * 64
                        nc.vector.tensor_copy(
                            out=zt[q][j % 2][lo:lo + 64,
                                             n * 512:(n + 1) * 512],
                            in_=mixbf[lo:lo + 64, j,
                                      n * 512:(n + 1) * 512],
                        )

            def prep_xl(j):
                """xl staging (PE transposes + masked evacuation) for pair j."""
                xlt = xlts[j % 2]
                for i2 in range(NT // 2):
                    pt = ps_sm.tile([128, 2, 128], BF16, name="pt", bufs=1)
                    for v in range(2):
                        i = 2 * i2 + v
                        nc.tensor.transpose(
                            pt[:, v, :], mixbf[:, j, i * 128:(i + 1) * 128],
                            identbf,
                        )
                    # [128, 2, 2, 64] masked-scaled evacuation into xl
                    nc.vector.tensor_mul(
                        xlt[:, 2 * i2:2 * i2 + 2, :, 0:DH],
                        pt[:, 0:2, :].rearrange("p v (q d) -> p v q d", q=2),
                        emask[:, 2 * i2:2 * i2 + 2, None, None].broadcast_to(
                            (128, 2, 2, DH)),
                    )

            def scores_chunk(j, q, i, ut, split_exp=False):
                """Banded scores + exp for t-chunk i of head (j, q): only
                columns >= C0[i]; the skipped tiles of other chunks are
                produced here by transposing this chunk's columns into ut.
                i == 6 computes chunks 6 AND 7 (shared band, one ACTIVATE).
                Returns the u tile(s)."""
                c0 = C0[i]
                zs = zt[q][j % 2]
                if i == 6:
                    psc = ps_big.tile([128, 2, 512], F32, name="psc")
                    for v in range(2):
                        nc.tensor.matmul(
                            psc[:, v, :],
                            lhsT=zs[:, (6 + v) * 128:(7 + v) * 128],
                            rhs=mixbf[:, j, 512:],
                            start=True,
                            stop=True,
                        )
                    u2 = upool.tile([128, 2, S], BF16, name="u2", bufs=3)
                    nc.scalar.activation(
                        out=u2[:, :, 512:], in_=psc, func=EXP, scale=0.125,
                    )
                    return [u2[:, 0, :], u2[:, 1, :]]
                psc = ps_big.tile([128, S], F32, name="psc")
                for n in range(2):
                    lo = max(c0, n * 512)
                    hi = (n + 1) * 512
                    if lo >= hi:
                        continue
                    nc.tensor.matmul(
                        psc[:, lo:hi],
                        lhsT=zs[:, i * 128:(i + 1) * 128],
                        rhs=mixbf[:, j, lo:hi],
                        start=True,
                        stop=True,
                    )
                u = upool.tile([128, S], BF16, name="u")
                if split_exp:
                    nc.scalar.activation(
                        out=u[:, c0:512], in_=psc[:, c0:512], func=EXP,
                        scale=0.125,
                    )
                    nc.scalar.activation(
                        out=u[:, 512:], in_=psc[:, 512:], func=EXP,
                        scale=0.125,
                    )
                else:
                    nc.scalar.activation(
                        out=u[:, c0:], in_=psc[:, c0:], func=EXP, scale=0.125,
                    )
                # U^T tiles for the chunks whose exp skips column-block i
                dsts = [c for c in range(NT) if 128 * i < C0[c]]
                if dsts:
                    pt = ps_sm.tile([128, NT, 128], BF16, name="pt", bufs=1)
                    for m, c in enumerate(dsts):
                        nc.tensor.transpose(
                            pt[:, m, :], u[:, c * 128:(c + 1) * 128], identbf)
                    m0 = IDX[(i, dsts[0])]
                    nc.vector.tensor_copy(
                        out=ut[:, m0:m0 + len(dsts), :],
                        in_=pt[:, 0:len(dsts), :])
                return [u]

            def ctx_half(h, us, ut, pc, b):
                """ctx for bank-half b of head h: stationary-U matmuls
                accumulating 4 s-chunk psum slots [128, DH+1] over the 8
                t-chunks."""
                xlt = xlts[(h // 2) % 2]
                q = h % 2
                for k in range(4):
                    sc = 4 * b + k
                    for i in range(NT):
                        if 128 * sc < C0[i]:
                            lhsT = ut[:, IDX[(sc, i)], :]
                        else:
                            lhsT = us[i][:, sc * 128:(sc + 1) * 128]
                        nc.tensor.matmul(
                            pc[:, k, 0:DH + 1],
                            lhsT=lhsT,
                            rhs=xlt[:, i, q, :],
                            start=(i == 0),
                            stop=(i == NT - 1),
                        )

            def epilogue_half(h, pc, b, fengs=None):
                """Normalize 4 s-chunk slots of head h into stages; flush
                immediately when fengs is given (final head)."""
                den = rpool.tile([128, 4], F32, name="denh")
                nc.vector.tensor_copy(out=den, in_=pc[:, :, DH])
                rcp = rpool.tile([128, 4], F32, name="rcph")
                nc.vector.reciprocal(out=rcp, in_=den)
                nc.vector.tensor_mul(
                    stages[:, 4 * b:4 * b + 4, h, :],
                    pc[:, :, 0:DH],
                    rcp[:, :, None].broadcast_to((128, 4, DH)),
                )
                if fengs is not None:
                    for k in range(4):
                        sc = b * 4 + k
                        fengs[sc % 3].dma_start(
                            out=out_d[sc * 128:(sc + 1) * 128,
                                      h * 64:(h + 1) * 64],
                            in_=stages[:, sc, h:h + 1, :],
                        )

            def ctx_epilogue(h, us, ut, fengs=None):
                for b in range(2):
                    pc = ps_ctx.tile([128, 4, 128], F32, name="pc")
                    ctx_half(h, us, ut, pc, b)
                    epilogue_half(h, pc, b, fengs)

            def flush(h0, h1):
                engs = [nc.sync, nc.gpsimd]
                for sj in range(NT):
                    engs[sj % len(engs)].dma_start(
                        out=out_d[sj * 128:(sj + 1) * 128, h0 * 64:h1 * 64],
                        in_=stages[:, sj, h0:h1, :],
                    )

            prep_proj(0)
            first_xl = True
            pending = None  # (h, q, us, ut) awaiting ctx + epilogue
            for j in range(NP):
                for q in range(2):
                    h = 2 * j + q
                    last = (h == 2 * NP - 1)
                    if last:
                        # Final head: emit its scores/exps, drain the pending
                        # head's ctx while they run, then this head's ctx +
                        # epilogue trail the last ACTIVATE.
                        ut = utpool.tile([128, NTR, 128], BF16, name="ut")
                        us = []
                        for i in range(7):
                            us.extend(scores_chunk(j, q, i, ut))
                        ph, pq, pus, put = pending
                        ctx_epilogue(ph, pus, put)
                        flush(6, 11)
                        fengs = [nc.sync, nc.gpsimd, nc.scalar]
                        ctx_epilogue(h, us, ut, fengs)
                        continue
                    ut = utpool.tile([128, NTR, 128], BF16, name="ut")
                    us = []
                    for i in range(6):
                        us.extend(scores_chunk(j, q, i, ut,
                                               split_exp=(h == 0 and i == 0)))
                    if first_xl:
                        # Deferred from prep(0): keeps the first head's
                        # scores off the PE critical path at startup.
                        prep_xl(0)
                        first_xl = False
                    if q == 1 and j + 1 < NP:
                        # Emit the next pair's prep mid-head: the PE gets the
                        # projection work while ACT still has queued exps,
                        # and the next head's scores are ready the moment
                        # this head's exps drain.
                        prep_proj(j + 1)
                        prep_xl(j + 1)
                    us.extend(scores_chunk(j, q, 6, ut))
                    if pending is not None:
                        ph, pq, pus, put = pending
                        ctx_epilogue(ph, pus, put)
                        if ph == 5:
                            flush(0, 6)
                    pending = (h, q, us, ut)

    nc.compile()
    return nc


def kernel(x, attention_mask, W, b, _profile=None):
    global _CACHED_NC
    if _CACHED_NC is None:
        _CACHED_NC = build_nc()
    nc = _CACHED_NC

    x = np.asarray(x, dtype=np.float32)
    attention_mask = np.asarray(attention_mask, dtype=np.float32)
    W = np.asarray(W, dtype=np.float32)
    b = np.asarray(b, dtype=np.float32)

    import ml_dtypes

    # Partition-contiguous repacks (see build_nc): [p, k*cols+c] = T[128k+p, c]
    wT = W.T.astype(ml_dtypes.bfloat16).reshape(NP, 128, D)
    w0 = np.ascontiguousarray(
        wT[:, :, 0:128].transpose(1, 0, 2).reshape(128, NP * 128))
    w1 = np.ascontiguousarray(
        wT[:, :, 128:D].transpose(1, 0, 2).reshape(128, NP * 640))
    bias_cols = np.ascontiguousarray(b.reshape(NP, 128).T)
    ident = np.eye(128, dtype=ml_dtypes.bfloat16)

    in_maps = []
    for i in range(B):
        xr = np.ascontiguousarray(
            x[i].T.astype(ml_dtypes.bfloat16).reshape(NP, 128, S)
            .transpose(1, 0, 2).reshape(128, NP * S))
        in_maps.append({
            "xr": xr,
            "w0": w0,
            "w1": w1,
            "bias_d": bias_cols,
            "mask_d": np.ascontiguousarray(
                attention_mask[i, 0, 0].reshape(NT, 128).T
            ),
            "ident_d": ident,
        })

    kwargs = dict(_profile) if _profile else {}
    res = run_bass_kernel_spmd(nc, in_maps, core_ids=list(range(B)), **kwargs)
    out = np.stack(
        [res.results[i]["out"].astype(np.float32) for i in range(B)], axis=0)
    if _profile:
        kernel.last_results = res
    return out


if __name__ == "__main__":
    rng = np.random.default_rng(0)
    x = rng.standard_normal((B, S, D), dtype=np.float32)
    m = np.zeros((B, 1, 1, S), dtype=np.float32)
    W = (rng.standard_normal((D, D), dtype=np.float32) / np.sqrt(D)).astype(np.float32)
    b = np.zeros((D,), dtype=np.float32)
    out = kernel(x, m, W, b)
    print("out", out.shape, out.dtype)
